# revision 12
# baseline (speedup 1.0000x reference)
"""Multihead attention (custom freq-bias) Trainium2 Bass kernel — v2.

Full inputs -> shard across 8 NeuronCores -> SPMD bass kernel -> host combine.

Sharding: core c handles batch b = c//2 and head-half s = c%2 (8 of 16 heads).
Heads are processed in 4 PAIRS per core; per pair:
  - qk projections emit qT/kT laid out [128 ch (2 heads x 64), 2048 q] bf16.
  - QK^T matmuls are ROW-TILED: head A uses PE rows 0-63, head B rows 64-127,
    issued back-to-back so both stream concurrently (2x effective).
  - exp runs on ScalarE as N=1024 activations (freq bias folded in as the
    per-partition activation bias), output bf16 et tiles [128 k, 2048 q].
  - AV matmuls are COL-TILED: head A -> PSUM partitions 0-63, head B -> 64-127
    (2x effective). Accumulated over 16 k-tiles in 2 chunks of 8; chunks are
    drained/accumulated into yT by DVE.
  - softmax denominators: DVE collapses the 16 et tiles into one etsum per
    head (sum over k-tile index), then M=1 matmuls with a ones column reduce
    over the 128 partitions; DVE reciprocal + a small select-matmul broadcast
    the per-(head,q) reciprocal across 128 partitions; DVE normalizes yT.
  - output projection is done per pair-GROUP {0,1} and {2,3}; each group's
    partial output goes to DRAM (bf16) and the host sums the two partials
    (plus bp) with the other core of the batch.
"""

import numpy as np
import ml_dtypes

import concourse.bass as bass
import concourse.tile as tile
from concourse import bacc, mybir

F32 = mybir.dt.float32
BF16 = mybir.dt.bfloat16
AF = mybir.ActivationFunctionType
ADD = mybir.AluOpType.add

B, N, C, H, D = 4, 2048, 1024, 16, 64
NCORES = 8
HC = C // 2          # 512 channels per core (8 heads x 64)
NKT = N // 128       # 16 key tiles
NPAIR = 4


def ts(i, sz):
    return slice(i * sz, (i + 1) * sz)


def kernel_body(ctx, tc, out, ins):
    """Per-core kernel. out: [4096, 1024] bf16 DRAM (2 group partials)."""
    nc = tc.nc
    xq, xk, xv = ins["xqt"], ins["xkt"], ins["xvt"]      # [1024, 2048] bf16
    wq, wk, wv = ins["wq"], ins["wk"], ins["wv"]          # [1024, 512] bf16
    wp = ins["wp"]                                        # [512, 1024] bf16
    bqc, bkc = ins["bqc"], ins["bkc"]                     # [128, 4] f32
    bvb = ins["bvb"]                                      # [128, 512] bf16
    freq = ins["freq"]                                    # [128, 16] f32
    singles = ctx.enter_context(tc.tile_pool(name="singles", bufs=1))

    # ---- persistent SBUF residents ----
    qT = [singles.tile([128, N], BF16, name=f"qT{p}") for p in range(NPAIR)]
    kT = [singles.tile([128, N], BF16, name=f"kT{p}") for p in range(NPAIR)]
    yT = [singles.tile([128, N], BF16, name=f"yT{p}") for p in range(NPAIR)]
    vt = [singles.tile([128, HC], BF16, name=f"v{i}") for i in range(NKT)]
    wp_sb = [singles.tile([128, C], BF16, name=f"wp{p}") for p in range(NPAIR)]
    bqc_sb = singles.tile([128, 4], F32, name="bqc")
    bkc_sb = singles.tile([128, 4], F32, name="bkc")
    bvb_sb = singles.tile([128, HC], BF16, name="bvb")
    freq_sb = singles.tile([128, NKT], F32, name="freq")
    onesf = singles.tile([128, 128], BF16, name="onesf")
    warm = singles.tile([1, 8], F32, name="warm")

    nc.sync.dma_start(out=bqc_sb, in_=bqc)
    nc.sync.dma_start(out=bkc_sb, in_=bkc)
    nc.sync.dma_start(out=bvb_sb, in_=bvb)
    nc.sync.dma_start(out=freq_sb, in_=freq)
    for p in range(NPAIR):
        nc.sync.dma_start(out=wp_sb[p], in_=wp[ts(p, 128), :])
    nc.vector.memset(onesf, 1.0)
    # warm up the exp table (ACT_TABLE_LOAD) off the critical path
    nc.vector.memset(warm, 0.0)
    nc.scalar.activation(out=warm, in_=warm, func=AF.Exp)

    # ---- pools ----
    xpool = ctx.enter_context(tc.tile_pool(name="xpool", bufs=10))
    wpool = ctx.enter_context(tc.tile_pool(name="wpool", bufs=8))
    etpool = ctx.enter_context(tc.tile_pool(name="etpool", bufs=16))
    espool = ctx.enter_context(tc.tile_pool(name="espool", bufs=3))
    otpool = ctx.enter_context(tc.tile_pool(name="otpool", bufs=4))
    rpool = ctx.enter_context(tc.tile_pool(name="rpool", bufs=3))
    psS = ctx.enter_context(tc.tile_pool(name="psS", bufs=1, space="PSUM"))
    psYp = ctx.enter_context(tc.tile_pool(name="psY", bufs=2, space="PSUM"))
    smalls = ctx.enter_context(tc.tile_pool(name="smalls", bufs=2, space="PSUM"))

    def qk_proj(group):
        """qT/kT for pairs (2g, 2g+1)."""
        for (x_d, w_d, b_sb, dst) in ((xq, wq, bqc_sb, qT), (xk, wk, bkc_sb, kT)):
            w_sb = []
            for k in range(8):
                t = wpool.tile([128, HC], BF16, tag="w", name=f"w{k}")
                nc.sync.dma_start(out=t, in_=w_d[ts(k, 128), :])
                w_sb.append(t)
            for qh in range(2):
                x_sb = []
                for k in range(8):
                    t = xpool.tile([128, 1024], BF16, tag="x", name=f"x{k}")
                    nc.sync.dma_start(out=t, in_=x_d[ts(k, 128), ts(qh, 1024)])
                    x_sb.append(t)
                for pair in (2 * group, 2 * group + 1):
                    for j in range(2):
                        ps = smalls.tile([128, 512], F32, tag="psml", name="psP")
                        for k in range(8):
                            nc.tensor.matmul(ps, w_sb[k][:, ts(pair, 128)],
                                             x_sb[k][:, ts(j, 512)],
                                             start=(k == 0), stop=(k == 7))
                        nc.vector.tensor_scalar(
                            dst[pair][:, qh * 1024 + j * 512:qh * 1024 + (j + 1) * 512],
                            ps, b_sb[:, pair:pair + 1], None, ADD)

    def v_proj():
        w_sb = []
        for k in range(8):
            t = wpool.tile([128, HC], BF16, tag="w", name=f"wv{k}")
            nc.sync.dma_start(out=t, in_=wv[ts(k, 128), :])
            w_sb.append(t)
        for qh in range(2):
            x_sb = []
            for k in range(8):
                t = xpool.tile([128, 1024], BF16, tag="x", name=f"xv{k}")
                nc.sync.dma_start(out=t, in_=xv[ts(k, 128), ts(qh, 1024)])
                x_sb.append(t)
            for ktl in range(8):
                kt = qh * 8 + ktl
                ps = smalls.tile([128, 512], F32, tag="psml", name="psV")
                for k in range(8):
                    nc.tensor.matmul(ps, x_sb[k][:, ts(ktl, 128)], w_sb[k],
                                     start=(k == 0), stop=(k == 7))
                nc.vector.tensor_tensor(vt[kt], ps, bvb_sb, ADD)

    def attention(pair):
        """QK^T + exp + AV + denominators + normalize for heads (2p, 2p+1)."""
        pa, pb = 0, 64  # partition offsets of head A / head B
        et = [[None, None] for _ in range(NKT)]       # [kt][head]
        es = [None, None]                              # etsum accum per head
        for chunk in range(2):
            for kt8 in range(8):
                kt = chunk * 8 + kt8
                for h, po in ((0, pa), (1, pb)):
                    t = etpool.tile([128, N], BF16, tag="et", name=f"et{h}")
                    et[kt][h] = t
                for qh in range(2):
                    psa = psS.tile([128, 1024], F32, tag="sa", name="psa")
                    psb = psS.tile([128, 1024], F32, tag="sb", name="psb")
                    for j in range(2):
                        qs = ts(qh * 2 + j, 512)
                        nc.tensor.matmul(psa[:, ts(j, 512)],
                                         kT[pair][0:64, ts(kt, 128)],
                                         qT[pair][0:64, qs],
                                         start=True, stop=True,
                                         skip_group_check=True)
                        nc.tensor.matmul(psb[:, ts(j, 512)],
                                         kT[pair][64:128, ts(kt, 128)],
                                         qT[pair][64:128, qs],
                                         start=True, stop=True,
                                         skip_group_check=True)
                    nc.scalar.activation(out=et[kt][0][:, ts(qh, 1024)], in_=psa,
                                         func=AF.Exp,
                                         bias=freq_sb[:, kt:kt + 1], scale=1.0)
                    nc.scalar.activation(out=et[kt][1][:, ts(qh, 1024)], in_=psb,
                                         func=AF.Exp,
                                         bias=freq_sb[:, kt:kt + 1], scale=1.0)
            # ---- AV for this chunk (col-tiled A/B), 4 q-chunks ----
            for qc in range(4):
                psy = psYp.tile([128, 512], F32, tag="psy", name="psy")
                for kt8 in range(8):
                    kt = chunk * 8 + kt8
                    nc.tensor.matmul(psy[0:64, :],
                                     vt[kt][:, pair * 128:pair * 128 + 64],
                                     et[kt][0][:, ts(qc, 512)],
                                     start=(kt8 == 0), stop=(kt8 == 7),
                                     skip_group_check=True)
                    nc.tensor.matmul(psy[64:128, :],
                                     vt[kt][:, pair * 128 + 64:pair * 128 + 128],
                                     et[kt][1][:, ts(qc, 512)],
                                     start=(kt8 == 0), stop=(kt8 == 7),
                                     skip_group_check=True)
                if chunk == 0:
                    nc.vector.tensor_copy(yT[pair][:, ts(qc, 512)], psy)
                else:
                    nc.vector.tensor_tensor(yT[pair][:, ts(qc, 512)],
                                            yT[pair][:, ts(qc, 512)], psy, ADD)
            # ---- collapse et -> etsum (DVE), sum over kt within chunk ----
            for h in range(2):
                if chunk == 0:
                    es[h] = espool.tile([128, N], BF16, tag="es", name=f"es{h}")
                    nc.vector.tensor_tensor(es[h], et[0][h], et[1][h], ADD)
                    rng = range(2, 8)
                else:
                    rng = range(8, 16)
                for kt in rng:
                    nc.vector.tensor_tensor(es[h], es[h], et[kt][h], ADD)
        # ---- denominators: ones[128,128] matmul = partition-reduce + broadcast
        # in one shot: psd[m, q] = sum_k es[k, q] for every m. Then DVE
        # reciprocal and normalize yT in place. Head A uses psd rows 0-63,
        # head B rows 64-127, so one psd per (qc) can serve both heads only
        # if dens differ per head -> one psd per (h, qc).
        for qc in range(4):
            for h in range(2):
                psd = smalls.tile([128, 512], F32, tag="psml", name="psd")
                nc.tensor.matmul(psd, onesf, es[h][:, ts(qc, 512)],
                                 start=True, stop=True, skip_group_check=True)
                # psd rows are all identical (= den_h); reciprocal_approx_fast
                # is broken on HW for base_partition != 0, so recip the full
                # tile at base 0 and use the h-half for the multiply.
                hsl = slice(h * 64, h * 64 + 64)
                rsb = rpool.tile([128, 512], F32, tag="rsb", name="rsb")
                nc.vector.reciprocal_approx_fast(out=rsb, in_=psd)
                nc.vector.tensor_mul(
                    yT[pair][hsl, ts(qc, 512)],
                    yT[pair][hsl, ts(qc, 512)], rsb[hsl, :])

    def out_proj(group):
        """Partial output for pairs (2g, 2g+1) -> DRAM rows [g*2048, (g+1)*2048)."""
        p0, p1 = 2 * group, 2 * group + 1
        for m in range(16):
            for n2 in range(2):
                ps = smalls.tile([128, 512], F32, tag="psml", name="psO")
                nc.tensor.matmul(ps, yT[p0][:, ts(m, 128)],
                                 wp_sb[p0][:, ts(n2, 512)], start=True, stop=False)
                nc.tensor.matmul(ps, yT[p1][:, ts(m, 128)],
                                 wp_sb[p1][:, ts(n2, 512)], start=False, stop=True)
                ot = otpool.tile([128, 512], BF16, tag="ot", name="ot")
                if group == 0:
                    nc.vector.tensor_copy(ot, ps)
                else:
                    nc.scalar.copy(ot, ps)
                nc.sync.dma_start(
                    out=out[group * N + m * 128:group * N + (m + 1) * 128,
                            ts(n2, 512)],
                    in_=ot)

    for group in range(2):
        qk_proj(group)
        if group == 0:
            v_proj()
        attention(2 * group)
        attention(2 * group + 1)
        out_proj(group)

    if "dbg" in ins:
        for p in range(NPAIR):
            nc.sync.dma_start(out=ins["dbg"][ts(p, 128), :], in_=yT[p])


INPUT_SPECS = {
    "xqt": ([C, N], BF16), "xkt": ([C, N], BF16), "xvt": ([C, N], BF16),
    "wq": ([C, HC], BF16), "wk": ([C, HC], BF16), "wv": ([C, HC], BF16),
    "wp": ([HC, C], BF16),
    "bqc": ([128, 4], F32), "bkc": ([128, 4], F32),
    "bvb": ([128, HC], BF16),
    "freq": ([128, NKT], F32),
}


def build_nc():
    from contextlib import ExitStack
    nc = bacc.Bacc("TRN2", target_bir_lowering=False, debug=False)
    ins = {name: nc.dram_tensor(name, shape, dt, kind="ExternalInput").ap()
           for name, (shape, dt) in INPUT_SPECS.items()}
    out = nc.dram_tensor("out", [2 * N, C], BF16, kind="ExternalOutput").ap()
    with tile.TileContext(nc) as tc:
        with ExitStack() as ctx:
            kernel_body(ctx, tc, out, ins)
    nc.compile()
    return nc


def make_freq():
    fr = np.linspace(0.0, 1.0, N, dtype=np.float32)
    fb = -((fr - 0.5) ** 2) * 10.0
    return np.ascontiguousarray(fb.reshape(NKT, 128).T).astype(np.float32)


def make_shards(inputs):
    """Full inputs -> list of 8 per-core input dicts."""
    q = np.asarray(inputs["query"], np.float32)
    k = np.asarray(inputs["key"], np.float32)
    v = np.asarray(inputs["value"], np.float32)
    Wq = np.asarray(inputs["Wq"], np.float32); bq = np.asarray(inputs["bq"], np.float32)
    Wk = np.asarray(inputs["Wk"], np.float32); bk = np.asarray(inputs["bk"], np.float32)
    Wv = np.asarray(inputs["Wv"], np.float32); bv = np.asarray(inputs["bv"], np.float32)
    Wp = np.asarray(inputs["Wp"], np.float32)
    freq = make_freq()
    scale = np.float32(1.0 / np.sqrt(D))

    shards = []
    for c in range(NCORES):
        b, s = c // 2, c % 2
        cs = slice(s * HC, (s + 1) * HC)
        bq_s = (bq[cs] * scale).astype(np.float32)
        bk_s = bk[cs].astype(np.float32)
        sh = {
            "xqt": np.ascontiguousarray(q[b].T),
            "xkt": np.ascontiguousarray(k[b].T),
            "xvt": np.ascontiguousarray(v[b].T),
            "wq": np.ascontiguousarray(Wq[:, cs]) * scale,
            "wk": np.ascontiguousarray(Wk[:, cs]),
            "wv": np.ascontiguousarray(Wv[:, cs]),
            "wp": np.ascontiguousarray(Wp[cs, :]),
            "bqc": np.ascontiguousarray(bq_s.reshape(4, 128).T),
            "bkc": np.ascontiguousarray(bk_s.reshape(4, 128).T),
            "bvb": np.broadcast_to(bv[cs], (128, HC)).copy(),
            "freq": freq,
        }
        for kk, (shape, dt) in INPUT_SPECS.items():
            want = ml_dtypes.bfloat16 if dt == BF16 else np.float32
            sh[kk] = np.asarray(sh[kk]).astype(want)
        shards.append(sh)
    return shards


_NC_CACHE = None


def kernel(**inputs):
    global _NC_CACHE
    shards = make_shards(inputs)
    if _NC_CACHE is None:
        _NC_CACHE = build_nc()
    nc = _NC_CACHE
    from concourse import bass_utils
    res = bass_utils.run_bass_kernel_spmd(nc, shards, core_ids=list(range(NCORES)))
    bp = np.asarray(inputs["bp"], np.float32)
    outs = []
    for r in res.results:
        o = np.asarray(r["out"], dtype=np.float32)
        outs.append(o[0:N] + o[N:2 * N])
    full = np.stack([outs[2 * b] + outs[2 * b + 1] + bp[None, :]
                     for b in range(B)])
    return full.astype(np.float32)


# revision 18
# speedup vs baseline: 1.0411x; 1.0411x over previous
"""Multihead attention (custom freq-bias) Trainium2 Bass kernel — v2.

Full inputs -> shard across 8 NeuronCores -> SPMD bass kernel -> host combine.

Sharding: core c handles batch b = c//2 and head-half s = c%2 (8 of 16 heads).
Heads are processed in 4 PAIRS per core; per pair:
  - qk projections emit qT/kT laid out [128 ch (2 heads x 64), 2048 q] bf16.
  - QK^T matmuls are ROW-TILED: head A uses PE rows 0-63, head B rows 64-127,
    issued back-to-back so both stream concurrently (2x effective).
  - exp runs on ScalarE as N=1024 activations (freq bias folded in as the
    per-partition activation bias), output bf16 et tiles [128 k, 2048 q].
  - AV matmuls are COL-TILED: head A -> PSUM partitions 0-63, head B -> 64-127
    (2x effective). Accumulated over 16 k-tiles in 2 chunks of 8; chunks are
    drained/accumulated into yT by DVE.
  - softmax denominators: DVE collapses the 16 et tiles into one etsum per
    head (sum over k-tile index), then M=1 matmuls with a ones column reduce
    over the 128 partitions; DVE reciprocal + a small select-matmul broadcast
    the per-(head,q) reciprocal across 128 partitions; DVE normalizes yT.
  - output projection is done per pair-GROUP {0,1} and {2,3}; each group's
    partial output goes to DRAM (bf16) and the host sums the two partials
    (plus bp) with the other core of the batch.
"""

import numpy as np
import ml_dtypes

import concourse.bass as bass
import concourse.tile as tile
from concourse import bacc, mybir

F32 = mybir.dt.float32
BF16 = mybir.dt.bfloat16
AF = mybir.ActivationFunctionType
ADD = mybir.AluOpType.add

B, N, C, H, D = 4, 2048, 1024, 16, 64
NCORES = 8
HC = C // 2          # 512 channels per core (8 heads x 64)
NKT = N // 128       # 16 key tiles
NPAIR = 4


def ts(i, sz):
    return slice(i * sz, (i + 1) * sz)


def kernel_body(ctx, tc, out, ins):
    """Per-core kernel. out: [4096, 1024] bf16 DRAM (2 group partials)."""
    nc = tc.nc
    xq, xk, xv = ins["xqt"], ins["xkt"], ins["xvt"]      # [1024, 2048] bf16
    wq, wk, wv = ins["wq"], ins["wk"], ins["wv"]          # [1024, 512] bf16
    wp = ins["wp"]                                        # [512, 1024] bf16
    bqc, bkc = ins["bqc"], ins["bkc"]                     # [128, 4] f32
    bvb = ins["bvb"]                                      # [128, 512] bf16
    freq = ins["freq"]                                    # [128, 16] f32
    singles = ctx.enter_context(tc.tile_pool(name="singles", bufs=1))

    # ---- persistent SBUF residents ----
    qT = [singles.tile([128, N], BF16, name=f"qT{p}") for p in range(NPAIR)]
    kT = [singles.tile([128, N], BF16, name=f"kT{p}") for p in range(NPAIR)]
    yT = [singles.tile([128, N], BF16, name=f"yT{p}") for p in range(NPAIR)]
    vt = [singles.tile([128, HC], BF16, name=f"v{i}") for i in range(NKT)]
    wp_sb = [singles.tile([128, C], BF16, name=f"wp{p}") for p in range(NPAIR)]
    bqc_sb = singles.tile([128, 4], F32, name="bqc")
    bkc_sb = singles.tile([128, 4], F32, name="bkc")
    bvb_sb = singles.tile([128, HC], BF16, name="bvb")
    freq_sb = singles.tile([128, NKT], F32, name="freq")
    onesf = singles.tile([128, 128], BF16, name="onesf")
    warm = singles.tile([1, 8], F32, name="warm")

    nc.sync.dma_start(out=bqc_sb, in_=bqc)
    nc.sync.dma_start(out=bkc_sb, in_=bkc)
    nc.sync.dma_start(out=bvb_sb, in_=bvb)
    nc.sync.dma_start(out=freq_sb, in_=freq)
    for p in range(NPAIR):
        nc.sync.dma_start(out=wp_sb[p], in_=wp[ts(p, 128), :])
    nc.vector.memset(onesf, 1.0)
    # warm up the exp table (ACT_TABLE_LOAD) off the critical path
    nc.vector.memset(warm, 0.0)
    nc.scalar.activation(out=warm, in_=warm, func=AF.Exp)

    # ---- pools ----
    xpool = ctx.enter_context(tc.tile_pool(name="xpool", bufs=10))
    wpool = ctx.enter_context(tc.tile_pool(name="wpool", bufs=8))
    etpool = ctx.enter_context(tc.tile_pool(name="etpool", bufs=16))
    espool = ctx.enter_context(tc.tile_pool(name="espool", bufs=3))
    otpool = ctx.enter_context(tc.tile_pool(name="otpool", bufs=4))
    rpool = ctx.enter_context(tc.tile_pool(name="rpool", bufs=3))
    psS = ctx.enter_context(tc.tile_pool(name="psS", bufs=1, space="PSUM"))
    psYp = ctx.enter_context(tc.tile_pool(name="psY", bufs=2, space="PSUM"))
    smalls = ctx.enter_context(tc.tile_pool(name="smalls", bufs=2, space="PSUM"))

    def qk_proj(pair):
        """qT/kT for one head pair."""
        for (x_d, w_d, b_sb, dst) in ((xq, wq, bqc_sb, qT), (xk, wk, bkc_sb, kT)):
            w_sb = []
            for k in range(8):
                t = wpool.tile([128, 128], BF16, tag="w", name=f"w{k}")
                nc.sync.dma_start(out=t, in_=w_d[ts(k, 128), ts(pair, 128)])
                w_sb.append(t)
            for qh in range(2):
                x_sb = []
                for k in range(8):
                    t = xpool.tile([128, 1024], BF16, tag="x", name=f"x{k}")
                    nc.sync.dma_start(out=t, in_=x_d[ts(k, 128), ts(qh, 1024)])
                    x_sb.append(t)
                for j in range(2):
                    ps = smalls.tile([128, 512], F32, tag="psml", name="psP")
                    for k in range(8):
                        nc.tensor.matmul(ps, w_sb[k],
                                         x_sb[k][:, ts(j, 512)],
                                         start=(k == 0), stop=(k == 7))
                    nc.vector.tensor_scalar(
                        dst[pair][:, qh * 1024 + j * 512:qh * 1024 + (j + 1) * 512],
                        ps, b_sb[:, pair:pair + 1], None, ADD)

    def v_proj():
        w_sb = []
        for k in range(8):
            t = wpool.tile([128, HC], BF16, tag="w", name=f"wv{k}")
            nc.sync.dma_start(out=t, in_=wv[ts(k, 128), :])
            w_sb.append(t)
        for qh in range(2):
            x_sb = []
            for k in range(8):
                t = xpool.tile([128, 1024], BF16, tag="x", name=f"xv{k}")
                nc.sync.dma_start(out=t, in_=xv[ts(k, 128), ts(qh, 1024)])
                x_sb.append(t)
            for ktl in range(8):
                kt = qh * 8 + ktl
                ps = smalls.tile([128, 512], F32, tag="psml", name="psV")
                for k in range(8):
                    nc.tensor.matmul(ps, x_sb[k][:, ts(ktl, 128)], w_sb[k],
                                     start=(k == 0), stop=(k == 7))
                nc.vector.tensor_tensor(vt[kt], ps, bvb_sb, ADD)

    def attention(pair):
        """QK^T + exp + AV + denominators + normalize for heads (2p, 2p+1)."""
        pa, pb = 0, 64  # partition offsets of head A / head B
        et = [[None, None] for _ in range(NKT)]       # [kt][head]
        es = [None, None]                              # etsum accum per head
        for chunk in range(2):
            for kt8 in range(8):
                kt = chunk * 8 + kt8
                for h, po in ((0, pa), (1, pb)):
                    t = etpool.tile([128, N], BF16, tag="et", name=f"et{h}")
                    et[kt][h] = t
                for qh in range(2):
                    psa = psS.tile([128, 1024], F32, tag="sa", name="psa")
                    psb = psS.tile([128, 1024], F32, tag="sb", name="psb")
                    for j in range(2):
                        qs = ts(qh * 2 + j, 512)
                        nc.tensor.matmul(psa[:, ts(j, 512)],
                                         kT[pair][0:64, ts(kt, 128)],
                                         qT[pair][0:64, qs],
                                         start=True, stop=True,
                                         skip_group_check=True)
                        nc.tensor.matmul(psb[:, ts(j, 512)],
                                         kT[pair][64:128, ts(kt, 128)],
                                         qT[pair][64:128, qs],
                                         start=True, stop=True,
                                         skip_group_check=True)
                    nc.scalar.activation(out=et[kt][0][:, ts(qh, 1024)], in_=psa,
                                         func=AF.Exp,
                                         bias=freq_sb[:, kt:kt + 1], scale=1.0)
                    nc.scalar.activation(out=et[kt][1][:, ts(qh, 1024)], in_=psb,
                                         func=AF.Exp,
                                         bias=freq_sb[:, kt:kt + 1], scale=1.0)
            # ---- AV for this chunk (col-tiled A/B), 4 q-chunks ----
            for qc in range(4):
                psy = psYp.tile([128, 512], F32, tag="psy", name="psy")
                for kt8 in range(8):
                    kt = chunk * 8 + kt8
                    nc.tensor.matmul(psy[0:64, :],
                                     vt[kt][:, pair * 128:pair * 128 + 64],
                                     et[kt][0][:, ts(qc, 512)],
                                     start=(kt8 == 0), stop=(kt8 == 7),
                                     skip_group_check=True)
                    nc.tensor.matmul(psy[64:128, :],
                                     vt[kt][:, pair * 128 + 64:pair * 128 + 128],
                                     et[kt][1][:, ts(qc, 512)],
                                     start=(kt8 == 0), stop=(kt8 == 7),
                                     skip_group_check=True)
                if chunk == 0:
                    nc.vector.tensor_copy(yT[pair][:, ts(qc, 512)], psy)
                else:
                    nc.vector.tensor_tensor(yT[pair][:, ts(qc, 512)],
                                            yT[pair][:, ts(qc, 512)], psy, ADD)
            # ---- collapse et -> etsum (DVE), sum over kt within chunk ----
            for h in range(2):
                if chunk == 0:
                    es[h] = espool.tile([128, N], BF16, tag="es", name=f"es{h}")
                    nc.vector.tensor_tensor(es[h], et[0][h], et[1][h], ADD)
                    rng = range(2, 8)
                else:
                    rng = range(8, 16)
                for kt in rng:
                    nc.vector.tensor_tensor(es[h], es[h], et[kt][h], ADD)
        # ---- denominators: ones[128,128] matmul = partition-reduce + broadcast
        # in one shot: psd[m, q] = sum_k es[k, q] for every m. Then DVE
        # reciprocal and normalize yT in place. Head A uses psd rows 0-63,
        # head B rows 64-127, so one psd per (qc) can serve both heads only
        # if dens differ per head -> one psd per (h, qc).
        for qc in range(4):
            for h in range(2):
                psd = smalls.tile([128, 512], F32, tag="psml", name="psd")
                nc.tensor.matmul(psd, onesf, es[h][:, ts(qc, 512)],
                                 start=True, stop=True, skip_group_check=True)
                # psd rows are all identical (= den_h); reciprocal_approx_fast
                # is broken on HW for base_partition != 0, so recip the full
                # tile at base 0 and use the h-half for the multiply.
                hsl = slice(h * 64, h * 64 + 64)
                rsb = rpool.tile([128, 512], F32, tag="rsb", name="rsb")
                nc.vector.reciprocal_approx_fast(out=rsb, in_=psd)
                nc.vector.tensor_mul(
                    yT[pair][hsl, ts(qc, 512)],
                    yT[pair][hsl, ts(qc, 512)], rsb[hsl, :])

    def out_proj(group):
        """Partial output for pairs (2g, 2g+1) -> DRAM rows [g*2048, (g+1)*2048)."""
        p0, p1 = 2 * group, 2 * group + 1
        for m in range(16):
            for n2 in range(2):
                ps = smalls.tile([128, 512], F32, tag="psml", name="psO")
                nc.tensor.matmul(ps, yT[p0][:, ts(m, 128)],
                                 wp_sb[p0][:, ts(n2, 512)], start=True, stop=False)
                nc.tensor.matmul(ps, yT[p1][:, ts(m, 128)],
                                 wp_sb[p1][:, ts(n2, 512)], start=False, stop=True)
                ot = otpool.tile([128, 512], BF16, tag="ot", name="ot")
                if group == 0:
                    nc.vector.tensor_copy(ot, ps)
                else:
                    nc.scalar.copy(ot, ps)
                nc.sync.dma_start(
                    out=out[group * N + m * 128:group * N + (m + 1) * 128,
                            ts(n2, 512)],
                    in_=ot)

    # Emission order = scheduler priority: later pairs' projections are
    # emitted inside earlier pairs' (ScalarE-bound) attention stretches so
    # TensorE never drains; out_proj(0) overlaps attention(2)/(3).
    qk_proj(0)
    qk_proj(1)
    v_proj()
    attention(0)
    qk_proj(2)
    attention(1)
    out_proj(0)
    qk_proj(3)
    attention(2)
    attention(3)
    out_proj(1)

    if "dbg" in ins:
        for p in range(NPAIR):
            nc.sync.dma_start(out=ins["dbg"][ts(p, 128), :], in_=yT[p])


INPUT_SPECS = {
    "xqt": ([C, N], BF16), "xkt": ([C, N], BF16), "xvt": ([C, N], BF16),
    "wq": ([C, HC], BF16), "wk": ([C, HC], BF16), "wv": ([C, HC], BF16),
    "wp": ([HC, C], BF16),
    "bqc": ([128, 4], F32), "bkc": ([128, 4], F32),
    "bvb": ([128, HC], BF16),
    "freq": ([128, NKT], F32),
}


def build_nc():
    from contextlib import ExitStack
    nc = bacc.Bacc("TRN2", target_bir_lowering=False, debug=False)
    ins = {name: nc.dram_tensor(name, shape, dt, kind="ExternalInput").ap()
           for name, (shape, dt) in INPUT_SPECS.items()}
    out = nc.dram_tensor("out", [2 * N, C], BF16, kind="ExternalOutput").ap()
    with tile.TileContext(nc) as tc:
        with ExitStack() as ctx:
            kernel_body(ctx, tc, out, ins)
    nc.compile()
    return nc


def make_freq():
    fr = np.linspace(0.0, 1.0, N, dtype=np.float32)
    fb = -((fr - 0.5) ** 2) * 10.0
    return np.ascontiguousarray(fb.reshape(NKT, 128).T).astype(np.float32)


def make_shards(inputs):
    """Full inputs -> list of 8 per-core input dicts."""
    q = np.asarray(inputs["query"], np.float32)
    k = np.asarray(inputs["key"], np.float32)
    v = np.asarray(inputs["value"], np.float32)
    Wq = np.asarray(inputs["Wq"], np.float32); bq = np.asarray(inputs["bq"], np.float32)
    Wk = np.asarray(inputs["Wk"], np.float32); bk = np.asarray(inputs["bk"], np.float32)
    Wv = np.asarray(inputs["Wv"], np.float32); bv = np.asarray(inputs["bv"], np.float32)
    Wp = np.asarray(inputs["Wp"], np.float32)
    freq = make_freq()
    scale = np.float32(1.0 / np.sqrt(D))

    shards = []
    for c in range(NCORES):
        b, s = c // 2, c % 2
        cs = slice(s * HC, (s + 1) * HC)
        bq_s = (bq[cs] * scale).astype(np.float32)
        bk_s = bk[cs].astype(np.float32)
        sh = {
            "xqt": np.ascontiguousarray(q[b].T),
            "xkt": np.ascontiguousarray(k[b].T),
            "xvt": np.ascontiguousarray(v[b].T),
            "wq": np.ascontiguousarray(Wq[:, cs]) * scale,
            "wk": np.ascontiguousarray(Wk[:, cs]),
            "wv": np.ascontiguousarray(Wv[:, cs]),
            "wp": np.ascontiguousarray(Wp[cs, :]),
            "bqc": np.ascontiguousarray(bq_s.reshape(4, 128).T),
            "bkc": np.ascontiguousarray(bk_s.reshape(4, 128).T),
            "bvb": np.broadcast_to(bv[cs], (128, HC)).copy(),
            "freq": freq,
        }
        for kk, (shape, dt) in INPUT_SPECS.items():
            want = ml_dtypes.bfloat16 if dt == BF16 else np.float32
            sh[kk] = np.asarray(sh[kk]).astype(want)
        shards.append(sh)
    return shards


_NC_CACHE = None


def kernel(**inputs):
    global _NC_CACHE
    shards = make_shards(inputs)
    if _NC_CACHE is None:
        _NC_CACHE = build_nc()
    nc = _NC_CACHE
    from concourse import bass_utils
    res = bass_utils.run_bass_kernel_spmd(nc, shards, core_ids=list(range(NCORES)))
    bp = np.asarray(inputs["bp"], np.float32)
    outs = []
    for r in res.results:
        o = np.asarray(r["out"], dtype=np.float32)
        outs.append(o[0:N] + o[N:2 * N])
    full = np.stack([outs[2 * b] + outs[2 * b + 1] + bp[None, :]
                     for b in range(B)])
    return full.astype(np.float32)
